# revision 2
# baseline (speedup 1.0000x reference)
"""Trainium2 Bass kernel for nn_NodeModel (gnn_message_passing), v2.

Reference computation:
    agg = segment_sum(edge_attr, edge_index[0], N)   # [N, 64]
    h   = relu(concat([x, agg], 1) @ W1 + b1)        # [N, 256]
    out = h @ W2 + b2                                # [N, 64]

v5 = v4 + the pending MLP's matmuls interleave INTO the next
supertile's scatter chunk stream (layer1 early, layer2 late) so the ACT
relus overlap scatter matmuls and the PE never waits on them; output
DMAs batched per group-pair.

v4 = v3 + software-pipelined MLP: the MLP for supertile g issues AFTER
scatter(g+1)'s matmuls, so the PE never stalls at the queue head waiting
for the DVE PSUM->SBUF agg copy (engine queues are strict FIFO).

v3 = v2 + DMA/engine plumbing: 2-group contiguous edge DMAs, output on
the scalar HWDGE ring, bf16 outputs, one-hot build split DVE/GPSIMD.

v2 strategy (vs v1): destination-PAIRED scatter.
  * Host pairs up edges with the same destination node: each PE
    contraction row holds TWO edges (features in cols 0:64 / 64:128).
    A 128-row chunk now covers 256 edges, halving the LDWEIGHTS
    bandwidth per edge (and the stationary is [128,128] bf16, which
    enables the compiler's Fast Weight Load path).
  * The scatter matmul accumulates agg_even on PSUM partitions 0:64
    and agg_odd on 64:128.  No cross-partition add is needed: layer 1
    of the MLP uses a stacked stationary [W1a; W1a] so
    W1a.T@agg_even + W1a.T@agg_odd = W1a.T@agg falls out of the
    contraction.  The x part of layer 1 runs as a separate K=64
    accumulating matmul straight from a resident xT tile.
  * All MLP operands bf16 (weights, x, agg, h); accumulation fp32 in
    PSUM.  Edge data travels bf16 as before.
"""

import os
import sys
import heapq

for _p in ("/opt/trn_rl_repo", "/root/.axon_site/_ro/trn_rl_repo"):
    if os.path.isdir(_p) and _p not in sys.path:
        sys.path.insert(0, _p)

import numpy as np
import ml_dtypes
from contextlib import ExitStack

import concourse.bass as bass
import concourse.tile as tile
from concourse import bacc, mybir
from concourse.bass_utils import run_bass_kernel_spmd

F32 = mybir.dt.float32
BF16 = mybir.dt.bfloat16
BF = ml_dtypes.bfloat16

NCORES = 8
D = 64            # feature dim
H = 256           # hidden dim
O = 64            # output dim
W = 32            # nodes per window
CHUNK = 128       # pair rows per chunk (PE contraction dim)
PCHUNKS = 2       # chunks per window
PCAP = CHUNK * PCHUNKS   # 256 pair slots per window
G = 16            # windows per group (= one supertile, one edge DMA)
ST = G * W        # 512-node MLP supertile
TMP_ENGINE = os.environ.get("K_TMP_ENGINE", "vector")  # vector | scalar


class Cfg:
    def __init__(self, n_nodes, n_pairs, extra=0):
        wpc = max(
            (n_nodes + NCORES * W - 1) // (NCORES * W),
            int(np.ceil(n_pairs * 1.01 / (PCAP * NCORES))),
        ) + extra
        wpc = ((wpc + 2 * G - 1) // (2 * G)) * (2 * G)  # 2-supertile-align
        self.WPC = wpc                            # windows per core
        self.NPC = W * wpc                        # node slots per core
        self.NWIN = NCORES * wpc
        self.NGRP = wpc // G                      # groups (= supertiles)


# ----------------------------------------------------------------- host pack

class PackOverflow(Exception):
    pass


def _assign_nodes(loads_per_node, n_nodes, cfg):
    """Balanced node->slot map by per-node pair load; cap PCAP pairs/window."""
    order = np.argsort(-loads_per_node, kind="stable")
    nwin = cfg.NWIN
    heap = [(0, w) for w in range(nwin)]
    counts = np.zeros(nwin, np.int64)     # nodes per window
    loads = np.zeros(nwin, np.int64)      # pairs per window
    slot_of_node = np.full(n_nodes, -1, np.int64)
    for n in order:
        d = int(loads_per_node[n])
        while True:
            load, w = heapq.heappop(heap)
            if counts[w] < W:
                break
        slot_of_node[n] = w * W + counts[w]
        counts[w] += 1
        loads[w] = load + d
        if counts[w] < W:
            heapq.heappush(heap, (loads[w], w))
    if loads.max() > PCAP:
        raise PackOverflow(f"window overflow: {loads.max()} > {PCAP}")
    perm = np.full(nwin * W, -1, np.int64)
    perm[slot_of_node] = np.arange(n_nodes)
    return slot_of_node, perm


def _pack(x, edge_index, edge_attr, W1, b1, W2, b2, cfg):
    n_nodes = x.shape[0]
    n_edges = edge_attr.shape[0]
    row = np.asarray(edge_index[0], np.int64)
    deg = np.bincount(row, minlength=n_nodes)
    pdeg = (deg + 1) >> 1
    slot_of_node, perm = _assign_nodes(pdeg, n_nodes, cfg)

    # ---- node features, transposed + permuted, split per core, bf16
    slots = np.zeros((cfg.NWIN * W, D), np.float32)
    mask = perm >= 0
    slots[mask] = np.asarray(x, np.float32)[perm[mask]]
    xT = np.ascontiguousarray(
        slots.reshape(NCORES, cfg.NPC, D).transpose(0, 2, 1)).astype(BF)

    # ---- edges routed to (window, chunk, row, member) pair slots
    slot = slot_of_node[row]
    eorder = np.argsort(slot, kind="stable")
    slot_s = slot[eorder]
    cnt = np.bincount(slot, minlength=cfg.NWIN * W)
    starts = np.concatenate([[0], np.cumsum(cnt)[:-1]])
    r = np.arange(n_edges) - starts[slot_s]
    member = r & 1
    prank = r >> 1
    pcnt = (cnt + 1) >> 1
    pcnt_w = pcnt.reshape(cfg.NWIN, W)
    poff_w = np.zeros_like(pcnt_w)
    poff_w[:, 1:] = np.cumsum(pcnt_w, axis=1)[:, :-1]
    if (poff_w[:, -1] + pcnt_w[:, -1]).max() > PCAP:
        raise PackOverflow("pair positions exceed window capacity")
    pairpos = poff_w.reshape(-1)[slot_s] + prank     # in [0, PCAP)
    ci = pairpos >> 7
    rrow = pairpos & (CHUNK - 1)
    wi = slot_s // W

    estream = np.zeros((cfg.NWIN, PCHUNKS, CHUNK, 2, D), BF)
    estream[wi, ci, rrow, member] = \
        np.asarray(edge_attr, np.float32)[eorder].astype(BF)
    rstream = np.zeros((cfg.NWIN, PCHUNKS, CHUNK), BF)
    rstream[wi, ci, rrow] = (slot_s % W).astype(BF)

    # device edge layout: [core, NGRP/2, 128, 2, G*PCHUNKS, 128]
    # (two supertiles interleaved per chunk-row -> one contiguous DMA)
    edges = np.ascontiguousarray(
        estream.reshape(NCORES, cfg.NGRP // 2, 2, G, PCHUNKS, CHUNK, 2 * D)
        .transpose(0, 1, 5, 2, 3, 4, 6)
    ).reshape(NCORES, cfg.NGRP // 2, CHUNK, 2 * G * PCHUNKS, 2 * D)
    rels = np.ascontiguousarray(
        rstream.reshape(NCORES, cfg.WPC, PCHUNKS, CHUNK)
        .transpose(0, 3, 1, 2)
    ).reshape(NCORES, CHUNK, cfg.WPC * PCHUNKS)

    iota = np.ascontiguousarray(
        np.tile(np.arange(W, dtype=BF), (CHUNK, G * PCHUNKS, 1)))

    # weights (bf16): layer1 split into agg part (stacked twice so the
    # even/odd PSUM halves fold during contraction) and x part (K=64)
    W1f = np.asarray(W1, np.float32)                 # [128, 256]
    W1s = np.ascontiguousarray(
        np.concatenate([W1f[D:2 * D], W1f[D:2 * D]], axis=0)).astype(BF)
    W1x = np.ascontiguousarray(W1f[0:D]).astype(BF)  # [64, 256]
    W2p = np.ascontiguousarray(
        np.asarray(W2, np.float32).reshape(2, 128, O).transpose(1, 0, 2)
        .reshape(128, 2 * O)).astype(BF)             # [128, 128]
    b1T = np.ascontiguousarray(
        np.asarray(b1, np.float32).reshape(2, 128).T)      # [128, 2]
    b2c = np.asarray(b2, np.float32).reshape(O, 1)         # [64, 1]

    in_maps = []
    for c in range(NCORES):
        in_maps.append({
            "xT": xT[c], "edges": edges[c], "rels": rels[c], "iota": iota,
            "W1s": W1s, "W1x": W1x, "W2p": W2p, "b1T": b1T, "b2": b2c,
        })
    return in_maps, perm, mask


# -------------------------------------------------------------- device build

def build_nc(cfg, reps=1, skip=frozenset()):
    nc = bacc.Bacc("TRN2", target_bir_lowering=False, debug=False)
    ap_xT = nc.dram_tensor("xT", [D, cfg.NPC], BF16,
                           kind="ExternalInput").ap()
    ap_edges = nc.dram_tensor(
        "edges", [cfg.NGRP // 2, CHUNK, 2 * G * PCHUNKS, 2 * D], BF16,
        kind="ExternalInput").ap()
    ap_rels = nc.dram_tensor(
        "rels", [CHUNK, cfg.WPC * PCHUNKS], BF16, kind="ExternalInput").ap()
    ap_iota = nc.dram_tensor(
        "iota", [CHUNK, G * PCHUNKS, W], BF16, kind="ExternalInput").ap()
    ap_W1s = nc.dram_tensor("W1s", [2 * D, H], BF16, kind="ExternalInput").ap()
    ap_W1x = nc.dram_tensor("W1x", [D, H], BF16, kind="ExternalInput").ap()
    ap_W2p = nc.dram_tensor("W2p", [H // 2, 2 * O], BF16,
                            kind="ExternalInput").ap()
    ap_b1T = nc.dram_tensor("b1T", [H // 2, 2], F32,
                            kind="ExternalInput").ap()
    ap_b2 = nc.dram_tensor("b2", [O, 1], F32, kind="ExternalInput").ap()
    ap_out = nc.dram_tensor("outT", [O, cfg.NPC], BF16,
                            kind="ExternalOutput").ap()

    AF = mybir.ActivationFunctionType
    KC = G * PCHUNKS
    with tile.TileContext(nc) as tc, ExitStack() as ctx:
        consts = ctx.enter_context(tc.tile_pool(name="consts", bufs=1))
        epool = ctx.enter_context(tc.tile_pool(name="edges", bufs=3))
        opool = ctx.enter_context(tc.tile_pool(name="onehot", bufs=3))
        tpool = ctx.enter_context(tc.tile_pool(name="aggtmp", bufs=3))
        hpool = ctx.enter_context(tc.tile_pool(name="hid", bufs=4))
        ypool = ctx.enter_context(tc.tile_pool(name="yout", bufs=3))
        ps_a = ctx.enter_context(tc.tile_pool(name="ps_agg", bufs=2,
                                              space="PSUM"))
        ps_h = ctx.enter_context(tc.tile_pool(name="ps_h", bufs=4,
                                              space="PSUM"))
        ps_o = ctx.enter_context(tc.tile_pool(name="ps_o", bufs=2,
                                              space="PSUM"))

        xT = consts.tile([D, cfg.NPC], BF16)
        nc.scalar.dma_start(xT[:], ap_xT)
        rels = consts.tile([CHUNK, cfg.WPC * PCHUNKS], BF16)
        nc.scalar.dma_start(rels[:], ap_rels)
        iota = consts.tile([CHUNK, KC, W], BF16)
        nc.scalar.dma_start(iota[:], ap_iota)
        W1s = consts.tile([2 * D, H], BF16)
        nc.scalar.dma_start(W1s[:], ap_W1s)
        W1x = consts.tile([D, H], BF16)
        nc.scalar.dma_start(W1x[:], ap_W1x)
        W2t = consts.tile([H // 2, 2 * O], BF16)
        nc.scalar.dma_start(W2t[:], ap_W2p)
        b1T = consts.tile([H // 2, 2], F32)
        nc.scalar.dma_start(b1T[:], ap_b1T)
        b2t = consts.tile([O, 1], F32)
        nc.scalar.dma_start(b2t[:], ap_b2)

        state = {"y": None, "hs": None}

        def mlp_l1(g, tmp, hh):
            sl = slice(g * ST, (g + 1) * ST)
            h_ps = ps_h.tile([128, ST], F32, tag="h_ps")
            nc.tensor.matmul(h_ps[:], W1s[:, hh * 128:(hh + 1) * 128],
                             tmp[:], start=True, stop=False)
            nc.tensor.matmul(h_ps[:], W1x[:, hh * 128:(hh + 1) * 128],
                             xT[:, sl], start=False, stop=True)
            h_sb = hpool.tile([128, ST], BF16, tag="h_sb")
            nc.scalar.activation(h_sb[:], h_ps[:], AF.Relu,
                                 bias=b1T[:, hh:hh + 1])
            return h_sb

        def mlp_l2(g, hs, ngrp):
            o_ps = ps_o.tile([O, ST], F32)
            nc.tensor.matmul(o_ps[:], W2t[:, 0:O], hs[0][:],
                             start=True, stop=False)
            nc.tensor.matmul(o_ps[:], W2t[:, O:2 * O], hs[1][:],
                             start=False, stop=True)
            if g % 2 == 0:
                state["y"] = ypool.tile([O, 2 * ST], BF16, name="o_sb", tag="o_sb")
            o_sb = state["y"]
            c0 = (g % 2) * ST
            nc.scalar.activation(o_sb[:, c0:c0 + ST], o_ps[:], AF.Identity,
                                 bias=b2t[:])
            if g % 2 == 1:
                g0 = g - 1
                nc.scalar.dma_start(ap_out[:, g0 * ST:(g0 + 2) * ST],
                                    o_sb[:])

        def mlp_step(pending, wg, ngrp):
            """Issue a slice of the pending supertile's MLP at scatter
            checkpoint wg (called between scatter chunk matmuls)."""
            if pending is None:
                return
            g, tmp = pending
            if wg == 0:
                state["hs"] = [mlp_l1(g, tmp, 0)]
            elif wg == 4:
                state["hs"].append(mlp_l1(g, tmp, 1))
            elif wg == 12:
                mlp_l2(g, state["hs"], ngrp)

        for rep in range(reps):
            pending = None
            for gg in range(cfg.NGRP // 2):
                et = epool.tile([CHUNK, 2 * KC, 2 * D], BF16)
                if "edma" not in skip:
                    nc.sync.dma_start(et[:], ap_edges[gg])
                elif rep == 0 and gg == 0:
                    nc.vector.memset(et[:], 0.0)
                for half in range(2):
                    g = 2 * gg + half
                    if "tt" not in skip:
                        oh = opool.tile([CHUNK, KC, W], BF16)
                        rel_bc = (rels[:, g * KC:(g + 1) * KC]
                                  .unsqueeze(2).broadcast_to([CHUNK, KC, W]))
                        nc.vector.tensor_tensor(oh[:], iota[:], rel_bc,
                                                op=mybir.AluOpType.is_equal)
                    a_ps = ps_a.tile([2 * D, ST], F32)
                    if "mm" not in skip:
                        for wg in range(G):
                            if "mlp" not in skip:
                                mlp_step(pending, wg, cfg.NGRP)
                            for c in range(PCHUNKS):
                                k = half * KC + wg * PCHUNKS + c
                                nc.tensor.matmul(
                                    a_ps[:, wg * W:(wg + 1) * W],
                                    et[:, k, :], oh[:, wg * PCHUNKS + c, :],
                                    start=(c == 0),
                                    stop=(c == PCHUNKS - 1))
                    tmp = tpool.tile([2 * D, ST], BF16)
                    if "copy" not in skip:
                        if TMP_ENGINE == "vector":
                            nc.vector.tensor_copy(tmp[:], a_ps[:])
                        else:
                            nc.scalar.activation(tmp[:], a_ps[:], AF.Copy)
                    if "mlp" in skip:
                        continue
                    pending = (g, tmp)
            if pending is not None:
                for wg in (0, 4, 12):
                    mlp_step(pending, wg, cfg.NGRP)
    nc.compile()
    return nc


# ------------------------------------------------------------------- driver

_CACHE = {}


def prepare(inputs, reps=1, skip=frozenset()):
    x = np.asarray(inputs["x"])
    edge_index = np.asarray(inputs["edge_index"])
    edge_attr = np.asarray(inputs["edge_attr"])
    row = np.asarray(edge_index[0], np.int64)
    deg = np.bincount(row, minlength=x.shape[0])
    n_pairs = int(((deg + 1) >> 1).sum())
    for extra in (0, 16, 32, 64):
        cfg = Cfg(x.shape[0], n_pairs, extra=extra)
        try:
            in_maps, perm, mask = _pack(
                x, edge_index, edge_attr,
                inputs["W1"], inputs["b1"], inputs["W2"], inputs["b2"], cfg)
            break
        except PackOverflow:
            continue
    else:
        raise RuntimeError("could not pack edges into windows")
    key = (cfg.WPC, reps, tuple(sorted(skip)))
    if key not in _CACHE:
        _CACHE[key] = build_nc(cfg, reps=reps, skip=skip)
    return _CACHE[key], in_maps, cfg, perm, mask


def unpack_out(results, cfg, perm, mask, n_nodes):
    slots = np.concatenate(
        [np.asarray(r["outT"], np.float32).T for r in results], axis=0)
    y = np.zeros((n_nodes, O), np.float32)
    y[perm[mask]] = slots[mask]
    return y


def kernel(**inputs):
    nc, in_maps, cfg, perm, mask = prepare(inputs)
    res = run_bass_kernel_spmd(nc, in_maps, list(range(NCORES)))
    return unpack_out(res.results, cfg, perm, mask,
                      np.asarray(inputs["x"]).shape[0])
